# revision 24
# baseline (speedup 1.0000x reference)
"""Trainium2 Bass kernel for CoarseBlockAttention (bf16 pipeline, v2).

Reference computation (per batch b, with x: (C, H, W), C=512, H=W=64, S=4):
  x_avg  = 4x4 block means of x            -> (nb=256, C)  [unfold order bh*16+bw]
  Q = x_avg @ Wq.T + bq ; K = x_avg @ Wk.T + bk
  A = softmax(Q K^T / sqrt(C))             -> (256, 256)
  V = x_flat @ Wv.T + bv  (x_flat: flat row-major pixels, (4096, C))
  Vsum = V summed over groups of 16 consecutive flat pixels -> (256, C)
  out_small = A @ Vsum                     -> (256, C)
  out[c, p] = out_small[p // 16, c]        (repeat_interleave by 16)

Algebraic restructuring (exact):
  * Vsum = Xsum @ Wv.T + 16*bv (linearity); the bias column is constant under
    softmax rows, added at the end.
  * Q K^T -> xa (Wq^T Wk) xa^T + 1 (u . xa[m])^T with u = Wk^T bq; row-const
    terms cancel in softmax.  Block-mean and 1/sqrt(C) scalings folded in.

bf16 plan (tolerance 2e-2, measured ~4.4e-3):
  * x cast bf16, column-reordered on host: col((i,i2,dh,bh,q)) for pixel
    p = (4bh+dh)*64 + 16q + 4i2 + i.  Summing the 4 outer i-planes (DVE,
    contiguous 2x adds) gives s1 = sums-of-4-pixels laid out (i2,dh,bh,q).
  * Xsum never materializes: Vs accumulates matmuls with lhsT = s1 i2-plane
    slices directly (the i2-sum happens in PSUM).  This also keeps the PE
    busy every chunk so the HAM clock stays at 2.4 GHz.
  * xa = sum over dh-planes (GPSIMD pair adds + DVE final), stored in tree
    order t=(i,b,q); the canonical order pos=(q,b,i) is obtained for free
    with strided matmul APs.  pos(m) = (m//4%4)*64 + (m//16)*4 + m%4; its
    inverse is applied by the At PSUM->SBUF copy so the out matmul emits
    logical column order.
  * Softmax skips the max-subtraction (logits ~N(0,5.4); fp32 exp exact).
  * 16x output expansion: ACT duplicates each value +bias into bf16 pairs,
    DVE broadcasts pairs 8x as int32 (2x packed), DMA out bf16; host upcasts.
"""

import math
from contextlib import ExitStack

import numpy as np
import ml_dtypes

import concourse.bacc as bacc
import concourse.bass as bass
import concourse.mybir as mybir
import concourse.tile as tile
from concourse._compat import get_trn_type
from concourse.bass_utils import run_bass_kernel_spmd
from concourse.masks import make_identity

B, C, H, W, S = 8, 512, 64, 64, 4
HW = H * W          # 4096
NB = (H // S) * (W // S)  # 256
P = 128
KC = C // P         # 4 contraction/channel chunks
F32 = mybir.dt.float32
BF16 = mybir.dt.bfloat16
I32 = mybir.dt.int32
AF = mybir.ActivationFunctionType
NP_BF16 = ml_dtypes.bfloat16


def _kernel_body(tc: "tile.TileContext", ctx, out, xb, wpk, b16p):
    nc = tc.nc

    singles = ctx.enter_context(tc.tile_pool(name="singles", bufs=1))
    xpool = ctx.enter_context(tc.tile_pool(name="xpool", bufs=3))
    trpool = ctx.enter_context(tc.tile_pool(name="trpool", bufs=2))
    prpool = ctx.enter_context(tc.tile_pool(name="prpool", bufs=2))
    expool = ctx.enter_context(tc.tile_pool(name="expool", bufs=3))

    # Warm the ACT exp table during the DMA-in phase.
    dummy = singles.tile([P, 1], F32, name="dummy")
    nc.vector.memset(dummy, 0.0)
    nc.scalar.activation(dummy, dummy, AF.Exp)

    ident = singles.tile([P, P], BF16, name="ident")
    make_identity(nc, ident)
    ones1 = singles.tile([1, P], BF16, name="ones1")
    nc.vector.memset(ones1, 1.0)

    wp_sb = singles.tile([P, KC, 2 * C + 1], BF16, name="wp_sb")  # w2|wv|us
    b16_sb = singles.tile([P, KC], F32, name="b16_sb")
    xa_sb = singles.tile([P, KC, NB], BF16, name="xa_sb")    # beta order (m)
    xl_sb = singles.tile([P, KC, NB], BF16, name="xl_sb")    # logical order (n)

    psA = tc.alloc_tile_pool(name="psA", bufs=1, space="PSUM")
    g_ps = [psA.tile([P, NB], F32, name=f"g_ps{j}") for j in range(KC)]
    vs_ps = [psA.tile([P, C], F32, name=f"vs_ps{m}") for m in range(2)]
    cs_ps = psA.tile([1, NB], F32, name="cs_ps")
    warm_ps = psA.tile([P, P], BF16, name="warm_ps")

    # A few dummy transposes bridge the PE into the first chunk's matmuls
    # so the HAM clock is warm from the start (scheduler hoists these).
    for _ in range(6):
        nc.tensor.transpose(warm_ps, ident, ident)

    # Streaming phase: per channel chunk, one 1 MB x DMA + a 262 KB weight
    # slice (w2 cols | wv cols | us col).
    for k in range(KC):
        x_t = xpool.tile([P, HW], BF16, name="x_t")
        rows = slice(k * P, (k + 1) * P)
        if k == KC - 1:
            # Split the last chunk so its reduction tree starts before the
            # final bytes land (the tree is on the critical path here).
            nc.sync.dma_start(out=x_t[:, 0:2048], in_=xb[rows, 0:2048])
            nc.sync.dma_start(out=x_t[:, 2048:3072], in_=xb[rows, 2048:3072])
            nc.sync.dma_start(out=x_t[:, 3072:4096], in_=xb[rows, 3072:4096])
        else:
            nc.sync.dma_start(out=x_t, in_=xb[rows, :])
        nc.sync.dma_start(out=wp_sb[:, k, :], in_=wpk[rows, :])
        if k == 0:
            nc.sync.dma_start(out=b16_sb, in_=b16p)
        first, last = (k == 0), (k == KC - 1)
        with nc.allow_low_precision(reason="bf16 pipeline"):
            # Level 0: sum the four i-planes -> s1 = sums of 4 consecutive
            # pixels, laid out (i2:4, dh:4, bh:16, q:4).
            xv = x_t.rearrange("p (i u) -> p i u", i=4)
            t0 = trpool.tile([P, 1024], BF16, name="t0")
            t1 = trpool.tile([P, 1024], BF16, name="t1")
            s1 = trpool.tile([P, 1024], BF16, name="s1")
            nc.vector.tensor_add(t0, xv[:, 0, :], xv[:, 1, :])
            nc.vector.tensor_add(t1, xv[:, 2, :], xv[:, 3, :])
            nc.vector.tensor_add(s1, t0, t1)
            # Vs accumulation straight off s1 (i2-sum folded into PSUM).
            # s1 plane-inner layout v = q*64 + b*4 + dh IS the internal m
            # order beta(m) = (m%4)*64 + (m//16)*4 + (m//4)%4, so plain
            # 128-col slices are the correctly-ordered weights.
            for m in range(2):
                for i2 in range(4):
                    nc.tensor.matmul(
                        vs_ps[m],
                        lhsT=s1[:, i2 * 256 + m * P:i2 * 256 + (m + 1) * P],
                        rhs=wp_sb[:, k, 512:1024],
                        start=(first and i2 == 0),
                        stop=(last and i2 == 3),
                    )
            # xa: sum over dh.  s1 inner order is q*64 + dh*16 + b, so the
            # dh-sliced reads are contiguous 16-runs and the outputs land
            # contiguous in the internal m order beta(m) = (m%4)*64 +
            # ((m//4)%4)*16 + m//16.  A second copy in logical order (pos
            # 16b + 4q2 + i2, scattered dst) feeds the L lhsT so the n axis
            # comes out un-permuted.
            s1v = s1.rearrange("p (i2 q dh b) -> p i2 q dh b", i2=4, q=4, dh=4)
            a0 = trpool.tile([P, 256], BF16, name="a0")
            a1 = trpool.tile([P, 256], BF16, name="a1")
            eng_a0 = nc.vector if last else nc.gpsimd
            eng_a0.tensor_add(
                a0.rearrange("p (i2 q b) -> p i2 q b", i2=4, q=4),
                s1v[:, :, :, 0, :], s1v[:, :, :, 1, :],
            )
            nc.gpsimd.tensor_add(
                a1.rearrange("p (i2 q b) -> p i2 q b", i2=4, q=4),
                s1v[:, :, :, 2, :], s1v[:, :, :, 3, :],
            )
            xl_dst = xl_sb[:, k, :].rearrange(
                "p (b q i2) -> p i2 q b", b=16, q=4, i2=4
            )
            nc.vector.tensor_add(
                xl_dst,
                a0.rearrange("p (i2 q b) -> p i2 q b", i2=4, q=4),
                a1.rearrange("p (i2 q b) -> p i2 q b", i2=4, q=4),
            )
            nc.gpsimd.tensor_add(xa_sb[:, k, :], a0, a1)
        # G/cs consume xa in its native tree (beta) order; rhs stays flat.
        for j in range(KC):
            nc.tensor.matmul(
                g_ps[j],
                lhsT=wp_sb[:, k, j * P:(j + 1) * P],
                rhs=xa_sb[:, k, :],
                start=first,
                stop=last,
            )
        nc.tensor.matmul(
            cs_ps,
            lhsT=wp_sb[:, k, 2 * C:2 * C + 1],
            rhs=xa_sb[:, k, :],
            start=first,
            stop=last,
        )

    with nc.allow_low_precision(reason="bf16 pipeline"):
        # PSUM -> SBUF staging, split across DVE and ACT (cs first: it gates
        # the last accumulation into each logits group).
        cs_sb = singles.tile([1, NB], BF16, name="cs_sb")
        nc.vector.tensor_copy(cs_sb, cs_ps)
        g_sb = singles.tile([P, KC, NB], BF16, name="g_sb")
        for j in range(KC):
            if j % 2 == 0:
                nc.vector.tensor_copy(g_sb[:, j, :], g_ps[j])
            else:
                nc.scalar.copy(g_sb[:, j, :], g_ps[j])
        vs_sb = singles.tile([P, 2, C], BF16, name="vs_sb")
        nc.scalar.copy(vs_sb[:, 0, :], vs_ps[0])
        nc.vector.tensor_copy(vs_sb[:, 1, :], vs_ps[1])
        psA.release()

        psB = tc.alloc_tile_pool(name="psB", bufs=1, space="PSUM")

        # Logits (both row chunks first, keeping the PE stream dense).
        a_sb = singles.tile([P, 2, NB], BF16, name="a_sb")
        at_sb = singles.tile([P, 2, NB], BF16, name="at_sb")
        rsum = singles.tile([P, 2], F32, name="rsum")
        l_ps = [psB.tile([P, NB], F32, name=f"l_ps{n}") for n in range(2)]
        for n in range(2):
            for k in range(KC):
                nc.tensor.matmul(
                    l_ps[n],
                    lhsT=xl_sb[:, k, n * P:(n + 1) * P],
                    rhs=g_sb[:, k, :],
                    start=(k == 0),
                    stop=False,
                )
            nc.tensor.matmul(
                l_ps[n], lhsT=ones1, rhs=cs_sb, start=False, stop=True
            )
        # Keep the PE HAM clock warm while the softmax runs on ACT/DVE
        # (scratch transposes into the t_ps rotation).
        for _ in range(6):
            wt = psB.tile([P, P], BF16, name="t_ps", bufs=2)
            nc.tensor.transpose(wt, ident, ident)
        # Softmax (no max subtraction) + transpose.  Rows of a_sb are
        # logical n (via xl), so the At copies are plain contiguous.
        for n in range(2):
            nc.scalar.activation(
                a_sb[:, n, :], l_ps[n], AF.Exp,
                accum_out=rsum[:, n:n + 1],
            )
            nc.vector.reciprocal(rsum[:, n:n + 1], rsum[:, n:n + 1])
            nc.vector.tensor_scalar_mul(
                a_sb[:, n, :], a_sb[:, n, :], rsum[:, n:n + 1]
            )
            for m in range(2):
                t_ps = psB.tile([P, P], BF16, name="t_ps", bufs=2)
                nc.tensor.transpose(
                    t_ps, a_sb[:, n, m * P:(m + 1) * P], ident
                )
                nc.vector.tensor_copy(
                    at_sb[:, m, n * P:(n + 1) * P], t_ps
                )

        # outT[o, n] = sum_m Vs[m, o] At[m, n]; +16*bv; 16x expansion.
        # Processed per n-half so the first out DMAs launch while the
        # second half's softmax/transpose still runs.
        o_ps = [psB.tile([P, NB], F32, name=f"o_ps{j}") for j in range(KC)]
        for h in range(2):
            cols = slice(h * P, (h + 1) * P)
            for j in range(KC):
                for m in range(2):
                    nc.tensor.matmul(
                        o_ps[j][:, cols],
                        lhsT=vs_sb[:, m, j * P:(j + 1) * P],
                        rhs=at_sb[:, m, cols],
                        start=(m == 0),
                        stop=(m == 1),
                    )
                # +bias, duplicate each value into a bf16 pair (ACT/DVE).
                paired = prpool.tile([P, 2 * P], BF16, name="paired")
                pview = paired.rearrange("p (q two) -> p q two", two=2)
                osrc = o_ps[j][:, cols].broadcast_to((P, P, 2))
                if (j + h) % 2 == 0:
                    nc.scalar.activation(
                        pview, osrc, AF.Identity, bias=b16_sb[:, j:j + 1]
                    )
                else:
                    nc.vector.tensor_scalar_add(
                        pview, osrc, b16_sb[:, j:j + 1]
                    )
                # DVE: broadcast pairs 8x as int32 (2x packed mode).
                ex = expool.tile([P, HW // 2], BF16, name="ex")
                nc.vector.tensor_copy(
                    ex.bitcast(I32).rearrange("p (q s) -> p q s", s=8),
                    paired.bitcast(I32).broadcast_to((P, P, 8)),
                )
                nc.sync.dma_start(
                    out=out[j * P:(j + 1) * P, h * 2048:(h + 1) * 2048],
                    in_=ex,
                )
        psB.release()


def _build():
    nc = bacc.Bacc(
        get_trn_type() or "TRN2", target_bir_lowering=False, debug=False
    )
    xb = nc.dram_tensor("xb", (C, HW), BF16, kind="ExternalInput").ap()
    wpk = nc.dram_tensor("wpk", (C, 2 * C + 1), BF16, kind="ExternalInput").ap()
    b16p = nc.dram_tensor("b16p", (P, KC), F32, kind="ExternalInput").ap()
    out = nc.dram_tensor("out", (C, HW), BF16, kind="ExternalOutput").ap()

    with tile.TileContext(nc) as tc:
        with ExitStack() as ctx:
            _kernel_body(tc, ctx, out, xb, wpk, b16p)
    nc.compile()
    return nc


_CACHE: dict = {}


def _get_nc():
    if "nc" not in _CACHE:
        _CACHE["nc"] = _build()
    return _CACHE["nc"]


def _prep_inputs(x, Wq, bq, Wk, bk, Wv, bv):
    f = lambda a: np.ascontiguousarray(np.asarray(a, dtype=np.float32))
    x, Wq, bq, Wk, bk, Wv, bv = map(f, (x, Wq, bq, Wk, bk, Wv, bv))
    s = 1.0 / math.sqrt(C)
    w2t = (Wk.T @ Wq) * (s / 256.0)
    usv = (Wk.T @ bq) * (s / 16.0)
    # Per-row pack: [w2 row | wv row | us] so each chunk is one contiguous DMA.
    wpk = np.concatenate([w2t, Wv.T, usv[:, None]], axis=1).astype(NP_BF16)
    b16p = np.ascontiguousarray(
        (16.0 * bv).reshape(KC, P).T.astype(np.float32)
    )
    # Column reorder: col((i,i2,q,dh,bh)) <- pixel (4bh+dh)*64 + 16q + 4i2 + i
    xr = (
        x.reshape(B, C, 16, 4, 4, 4, 4)       # (b, c, bh, dh, q, i2, i)
        .transpose(0, 1, 6, 5, 4, 3, 2)        # (b, c, i, i2, q, dh, bh)
        .reshape(B, C, HW)
        .astype(NP_BF16)
    )
    in_maps = [
        {"xb": np.ascontiguousarray(xr[b]), "wpk": wpk, "b16p": b16p}
        for b in range(B)
    ]
    return in_maps


def run(inputs: dict, trace: bool = False, tmpdir: str | None = None):
    """Run on 8 NeuronCores; returns (output (B,C,H,W) f32, BassKernelResults)."""
    nc = _get_nc()
    in_maps = _prep_inputs(**inputs)
    rr = run_bass_kernel_spmd(nc, in_maps, list(range(B)), trace=trace, tmpdir=tmpdir)
    out = np.stack([np.asarray(r["out"]).astype(np.float32) for r in rr.results])
    return out.reshape(B, C, H, W), rr


def kernel(**inputs) -> np.ndarray:
    out, _ = run(inputs, trace=False)
    return out
